# revision 38
# baseline (speedup 1.0000x reference)
"""GAT (graph attention network) Bass kernel for 8 trn2 NeuronCores.

Strategy (row-sharding): core k owns query rows [k*512, (k+1)*512).
 - Every core computes Wh = x @ W[h] for ALL nodes (replicated, cheap on PE)
   in [node-on-partition, feature] layout; s1 = x@(W a1) and s2 = x@(W a2)
   come from one thin matmul against a packed [wa1|wa2] weight block.
 - Hidden attention per head, transposed layout [keys j on partitions,
   own rows i free]: z = nm + s1[i] + s2[j] assembled as one slab-wide
   tensor_tensor (2x bf16 mode, s1 broadcast via 0-stride AP) plus a
   per-block tensor_scalar (+s2, 4x mode); leaky split between ScalarE
   Prelu and a fused DVE STT max(z, a*z); exp on ScalarE over 8-block
   slabs; out_head.T = [Wh|1].T @ P accumulated in PSUM gives both
   att@Wh and softmax denominators.
 - Per-head normalize: 1/den via exp(-ln(den)) on ScalarE (reads PSUM
   directly), then mult + elu -> h kept transposed as lhsT for
   Wh_o = h @ W_out. AllGather of Wh_o partials is split into an early
   tiny s2o gather (unblocks the output layer's elementwise) and the
   main [Wh_o|s2o] gather; the output attention layer then runs the
   same way, followed by elu + log_softmax.
"""

import sys

sys.path.insert(0, "/opt/trn_rl_repo")

import numpy as np
import ml_dtypes

import concourse.bass as bass
import concourse.bacc as bacc
import concourse.tile as tile
from concourse import mybir
from concourse.bass_utils import run_bass_kernel_spmd
from concourse.masks import make_identity

F32 = mybir.dt.float32
BF16 = mybir.dt.bfloat16
BF = ml_dtypes.bfloat16
ADD = mybir.AluOpType.add
MULT = mybir.AluOpType.mult
MAX = mybir.AluOpType.max
AF = mybir.ActivationFunctionType

# problem shape (hardcoded per spec)
N = 4096
F_IN = 512
O = 64
H = 8
C = 16
N_CORES = 8
NEG = -100.0  # additive mask offset; exp(leaky(-100+e)) <= ~1e-8
ALPHA = 0.2

# knobs
GROUP = 8          # j-blocks per activation slab
D_LEAKY = 3        # blocks per slab with leaky on DVE (rest: ScalarE Prelu)
D_LEAKY_ALT = 4    # alternate-slab DVE-leaky count (balance fine-tune)
D_LEAKY_OUT = 5    # output layer DVE-leaky blocks
GP_ZTT = 0         # gpsimd z-base offload: SBUF-port contention with DVE
                   # 2-port modes makes this a net loss; keep 0

KF = F_IN // 128   # f_in k-tiles


def _build_nc(n_cores=N_CORES, n=N):
    NB = n // 128          # node blocks (keys)
    OWN = n // n_cores     # own query rows per core
    OB = OWN // 128        # own row blocks
    NG = NB // GROUP       # slabs per attention pass
    nc = bacc.Bacc("TRN2", target_bir_lowering=False, debug=False,
                   num_devices=n_cores)

    # Pin every activation to the natural_log_exp_and_others table set
    # (it contains all four functions we use: Prelu, Exp, Ln, Copy).
    # Default set selection assigns Prelu/Exp and Ln to different sets,
    # causing a ~2.7us ACT_TABLE_LOAD+drain on every per-head reciprocal.
    # Positions in the table list are preserved (walrus IDs are indices),
    # only membership is masked.
    import types
    import bass_rust as _bass_rust
    from concourse.hw_specs import get_activation_tables

    def _one_set_act_loads(self):
        has_activation = any(
            isinstance(i, mybir.InstActivation)
            for b in self.main_func.blocks
            for i in b.instructions)
        if not has_activation:
            return
        pin = "natural_log_exp_and_others"
        all_t = get_activation_tables(self.m.arch)
        pinned = all_t[pin]
        tables = [(name, fns if name == pin else fns - pinned)
                  for name, fns in all_t.items()]
        _bass_rust.insert_act_table_loads(self, tables)

    nc.insert_act_table_loads = types.MethodType(_one_set_act_loads, nc)

    # per-core external inputs (host-packed, see _pack_inputs)
    d_xT = nc.dram_tensor("xT", [128, KF * n], BF16, kind="ExternalInput")
    d_xo = nc.dram_tensor("xo", [128, KF * OWN], BF16, kind="ExternalInput")
    d_w64 = nc.dram_tensor("w64", [128, H * KF * O], BF16, kind="ExternalInput")
    d_waA = nc.dram_tensor("waA", [128, KF * 2 * H], BF16, kind="ExternalInput")
    d_wo1 = nc.dram_tensor("wo1r", [128, KF * 128], BF16, kind="ExternalInput")
    d_nm = nc.dram_tensor("nmT", [128, NB * OWN], BF16, kind="ExternalInput")
    d_wot = nc.dram_tensor("wot", [128, KF * C], BF16, kind="ExternalInput")
    d_a2o = nc.dram_tensor("a2o", [128, C], F32, kind="ExternalInput")
    d_out = nc.dram_tensor("out", [OWN, C], F32, kind="ExternalOutput")

    with tile.TileContext(nc) as tc:
        with (
            tc.tile_pool(name="dram", bufs=1, space="DRAM") as dram,
            tc.tile_pool(name="const", bufs=1) as const,
            tc.tile_pool(name="work", bufs=2) as work,
            tc.tile_pool(name="small", bufs=2) as small,
            tc.tile_pool(name="psA", bufs=3, space="PSUM") as psA,
            tc.tile_pool(name="psH", bufs=2, space="PSUM") as psH,
            tc.tile_pool(name="psM", bufs=2, space="PSUM") as psM,
        ):
            # ---- load inputs (priority order: phase-S feeders first) ----
            waA = const.tile([128, KF * 2 * H], BF16)
            nc.sync.dma_start(out=waA, in_=d_waA[:])
            xo = const.tile([128, KF * OWN], BF16)
            for c in range(0, KF * OWN, 1024):
                nc.sync.dma_start(out=xo[:, c: c + 1024],
                                  in_=d_xo[:, c: c + 1024])
            # xT loads split: the first 1024 node-columns of each k-tile
            # feed s-chunks 0-1 and head-0 Wh chunks 0-1, so they go first;
            # the bulk follows.
            PFX = 1024
            xT = const.tile([128, KF * n], BF16)
            for k in range(KF):
                nc.sync.dma_start(out=xT[:, k * n: k * n + PFX],
                                  in_=d_xT[:, k * n: k * n + PFX])
            w64 = const.tile([128, H * KF * O], BF16)
            w_ = 2 * KF * O
            nc.sync.dma_start(out=w64[:, 0: w_], in_=d_w64[:, 0: w_])
            nm = const.tile([128, NB * OWN], BF16)
            wg = GROUP * OWN

            def load_nm(g):
                # 4 sub-chunks land on 4 DMA queues: ~4x less latency than
                # one 1MB transfer on a single ~25GB/s queue
                for c in range(4):
                    nc.sync.dma_start(
                        out=nm[:, g * wg + c * wg // 4: g * wg + (c + 1) * wg // 4],
                        in_=d_nm[:, g * wg + c * wg // 4: g * wg + (c + 1) * wg // 4])

            load_nm(0)
            # column-window-major order: s-chunk/Wh consumers need a column
            # window across ALL k-tiles, so land windows in consumption
            # order, with the nm slab each window's attention needs between
            for w0 in range(PFX, n, PFX):
                for k in range(KF):
                    nc.sync.dma_start(out=xT[:, k * n + w0: k * n + w0 + PFX],
                                      in_=d_xT[:, k * n + w0: k * n + w0 + PFX])
                load_nm(w0 // PFX)
            for hh in range(2, H, 2):
                nc.sync.dma_start(out=w64[:, hh * KF * O: hh * KF * O + w_],
                                  in_=d_w64[:, hh * KF * O: hh * KF * O + w_])
            wo1 = const.tile([128, KF * 128], BF16)
            nc.sync.dma_start(out=wo1, in_=d_wo1[:])
            wot = const.tile([128, KF * C], BF16)
            nc.sync.dma_start(out=wot, in_=d_wot[:])
            a2o = const.tile([128, C], F32)
            nc.sync.dma_start(out=a2o, in_=d_a2o[:])

            ident = const.tile([128, 128], F32)
            make_identity(nc, ident[:])
            identb = const.tile([16, 16], BF16)
            make_identity(nc, identb[:])

            wht = [const.tile([128, NB * 65], BF16, tag=f"wh{h}", name=f"wh{h}")
                   for h in range(H)]
            # ones columns are memset lazily (first Wh chunk emission) on the
            # Pool queue: 8 strided memsets on VectorE would stall phase S
            wht_init = [False] * H
            hT_all = const.tile([128, KF * OWN], BF16)

            # ---- phase S: s1/s2 rows via thin matmuls ----
            # s1 for own rows -> broadcast to all partitions per head
            ps1 = psM.tile([16, OWN], F32, tag="mm", name="ps1")
            for k in range(KF):
                nc.tensor.matmul(ps1[:], waA[:, k * 16: (k + 1) * 16],
                                 xo[:, k * OWN: (k + 1) * OWN],
                                 start=(k == 0), stop=(k == KF - 1))
            s1T = const.tile([16, OWN], BF16)
            nc.vector.tensor_copy(s1T[:], ps1[:])
            # broadcast row h to 128 partitions via a DRAM bounce; these
            # ride ScalarE's HWDGE queue - the sync queue is backed up with
            # bulk input loads at this point and would add ~30us of latency
            s1d = dram.tile([8, OWN], BF16)
            nc.scalar.dma_start(out=s1d[:], in_=s1T[8:16, :])
            s1b = [const.tile([128, OWN], BF16, tag=f"s1b{h}", name=f"s1b{h}")
                   for h in range(H)]
            for h in range(H):
                nc.scalar.dma_start(out=s1b[h][:],
                                    in_=s1d[h: h + 1, :].to_broadcast([128, OWN]))
            # s2 rows for all nodes; s2f split per slab-group and emitted
            # lazily (slab g only needs chunks/transposes 2g, 2g+1) so the
            # first attention slab is not gated on the whole sweep
            NGG = NB // GROUP
            s2fg = [const.tile([128, GROUP * 8], F32, tag=f"s2f{g}",
                               name=f"s2f{g}") for g in range(NGG)]

            def emit_s_chunk(ch):
                pss = psM.tile([16, 512], F32, tag="mm", name=f"s12_{ch}")
                for k in range(KF):
                    nc.tensor.matmul(
                        pss[:], waA[:, k * 16: (k + 1) * 16],
                        xT[:, k * n + ch * 512: k * n + ch * 512 + 512],
                        start=(k == 0), stop=(k == KF - 1))
                # copies on ScalarE: it idles through the head-0 window, and
                # keeping these off VectorE's in-order queue stops late xT
                # windows from head-of-line-blocking the first z slabs
                s2c = small.tile([8, 512], BF16, tag="s2c", name=f"s2c{ch}")
                nc.scalar.activation(s2c[:], pss[0:8, :], AF.Copy)
                # transpose this chunk's 4 node-blocks into s2fg
                pst = psM.tile([128, 32], BF16, tag="s2t", name=f"s2t_{ch}",
                               bufs=1)
                for u in range(4):
                    nc.tensor.transpose(pst[:, u * 8: (u + 1) * 8],
                                        s2c[:, u * 128: (u + 1) * 128],
                                        identb[0:8, 0:8])
                jb = ch * 4
                nc.scalar.activation(
                    s2fg[jb // GROUP][:, (jb % GROUP) * 8: (jb % GROUP) * 8 + 32],
                    pst[:], AF.Copy)

            emit_s_chunk(0)
            emit_s_chunk(1)
            emit_s_chunk(2)
            emit_s_chunk(3)

            # ---- phase A helper: Wh for one head, one 4-block chunk;
            # 4 node-blocks per PSUM bank as sequential accumulation groups,
            # one batched cast. Emission is interleaved into phase B so the
            # in-order engine queues pipeline A(h+1) under B(h). ----
            def emit_wh_chunk(h, nb4):
                if not wht_init[h]:
                    wht_init[h] = True
                    nc.gpsimd.memset(
                        wht[h][:].rearrange("p (b w) -> p b w",
                                            w=65)[:, :, 64:65], 1.0)
                ps = psA.tile([128, 4 * O], F32, tag="whp",
                              name=f"whp_{h}_{nb4}")
                for sub in range(4):
                    nb = nb4 * 4 + sub
                    for k in range(KF):
                        nc.tensor.matmul(
                            ps[:, sub * O: (sub + 1) * O],
                            xT[:, k * n + nb * 128: k * n + nb * 128 + 128],
                            w64[:, (h * KF + k) * O: (h * KF + k) * O + O],
                            start=(k == 0), stop=(k == KF - 1))
                dst = (wht[h][:, nb4 * 4 * 65: (nb4 * 4 + 4) * 65]
                       .rearrange("p (b w) -> p b w", w=65)[:, :, 0:O])
                src = ps[:].rearrange("p (b w) -> p b w", w=O)
                # ScalarE copy: keeps the cast off VectorE's in-order queue
                # (Copy is present in every ACT table set - no table load)
                nc.scalar.activation(dst, src, AF.Copy)

            NB4 = NB // 4
            emit_wh_chunk(0, 0)
            if NB4 > 1:
                emit_wh_chunk(0, 1)

            # ---- attention slab helper (hidden + output layers) ----
            def attention(s2col, s1bt, lhsT_tile, lhsw, m_rows, psacc, tagp,
                          pre_slab=None, premade=None, d_leaky=(D_LEAKY,),
                          gp_ztt=GP_ZTT, s2col_pre=None, halves=False):
                for g in range(NG):
                    if pre_slab is not None:
                        pre_slab(g)
                    premade_g = premade and g in premade
                    if premade_g:
                        zs = premade[g]
                    else:
                        zs = work.tile([128, GROUP * OWN], BF16, tag="z",
                                       name=f"z{tagp}_{g}")
                        # slab-wide base: zs = nm + s1 (s1 broadcast across
                        # blocks via 0-stride AP); 2x bf16 TT mode.
                        nd = GROUP - gp_ztt
                        if nd > 0:
                            nc.vector.tensor_tensor(
                                zs[:, 0:nd * OWN]
                                .rearrange("p (o w) -> p o w", w=OWN),
                                nm[:, g * GROUP * OWN: (g * GROUP + nd) * OWN]
                                .rearrange("p (o w) -> p o w", w=OWN),
                                s1bt[:].rearrange("p (o w) -> p o w", o=1)
                                .to_broadcast([128, nd, OWN]),
                                ADD)
                        for q in range(nd, GROUP):
                            # trailing blocks off-loaded to the Pool engine
                            nc.gpsimd.tensor_tensor(
                                zs[:, q * OWN: (q + 1) * OWN],
                                nm[:, (g * GROUP + q) * OWN:
                                   (g * GROUP + q + 1) * OWN],
                                s1bt[:], ADD)
                    # per-block +s2 (per-partition scalar, 4x TS mode)
                    s2c_g = s2col_pre if (premade_g and s2col_pre) else s2col
                    us = work.tile([128, GROUP * OWN], BF16, tag="p",
                                   name=f"u{tagp}_{g}")
                    os_ = work.tile([128, GROUP * OWN], BF16, tag="o",
                                    name=f"o{tagp}_{g}")
                    d = d_leaky[g % len(d_leaky)]
                    # halves: process the slab as two 4-block pieces so the
                    # first matmuls (and the last exp) are ~4us earlier -
                    # shortens the latency-bound output tail
                    chunks = ([(0, GROUP // 2), (GROUP // 2, GROUP)]
                              if halves else [(0, GROUP)])
                    for ci, (q0, q1) in enumerate(chunks):
                        for q in range(q0, q1):
                            jb = g * GROUP + q
                            nc.vector.tensor_scalar(
                                zs[:, q * OWN: (q + 1) * OWN],
                                zs[:, q * OWN: (q + 1) * OWN],
                                s2c_g(jb), None, ADD)
                        # Prelu-blocks first, then DVE STT-leaky blocks;
                        # the split point lands inside one of the chunks
                        sp = (GROUP - d) * OWN
                        c0, c1 = q0 * OWN, q1 * OWN
                        pe_ = min(max(sp, c0), c1)
                        if pe_ > c0:
                            nc.scalar.activation(us[:, c0:pe_], zs[:, c0:pe_],
                                                 AF.Prelu, alpha=ALPHA)
                        if c1 > pe_:
                            # fused leaky on DVE: max(z, alpha*z) in one STT
                            nc.vector.scalar_tensor_tensor(
                                us[:, pe_:c1], zs[:, pe_:c1], ALPHA,
                                zs[:, pe_:c1], MULT, MAX)
                        # Exp in its own output slab: writing back into zs
                        # would extend zs's lifetime through the matmul reads
                        nc.scalar.activation(os_[:, c0:c1], us[:, c0:c1],
                                             AF.Exp)
                        for q in range(q0, q1):
                            jb = g * GROUP + q
                            nc.tensor.matmul(
                                psacc[0: m_rows, :],
                                lhsT_tile[:, jb * lhsw: jb * lhsw + m_rows],
                                os_[:, q * OWN: (q + 1) * OWN],
                                start=(jb == 0), stop=(jb == NB - 1))

            def emit_who_half(tag, cs):
                who = const.tile([128, OB * 17], F32, name=f"whoown{tag}")
                for ib in range(OB):
                    pw = psM.tile([128, OWN], F32, tag="mm",
                                  name=f"pw{tag}{ib}")
                    for ci, c in enumerate(cs):
                        nc.tensor.matmul(
                            pw[:, 0:C],
                            hT_all[:, c * OWN + ib * 128: c * OWN + ib * 128 + 128],
                            wot[:, c * C: (c + 1) * C],
                            start=(ci == 0), stop=(ci == len(cs) - 1))
                    nc.vector.tensor_copy(who[:, ib * 17: ib * 17 + C],
                                          pw[:, 0:C])
                    tmp = small.tile([128, C], F32, tag="s2tmp",
                                     name=f"s2o{tag}{ib}")
                    nc.vector.scalar_tensor_tensor(
                        tmp[:], pw[:, 0:C], 1.0, a2o[:], MULT, MULT,
                        accum_out=who[:, ib * 17 + 16: ib * 17 + 17])
                return who

            def emit_gather_start(tag, src_ap, width):
                ci = dram.tile([128, width], F32, name=f"cci{tag}")
                co = dram.tile([n_cores * 128, width], F32,
                               addr_space="Shared" if n_cores > 1 else "Local",
                               name=f"cco{tag}")
                nc.gpsimd.dma_start(out=ci[:], in_=src_ap)
                if n_cores > 1:
                    nc.gpsimd.collective_compute(
                        "AllGather", mybir.AluOpType.bypass,
                        replica_groups=[list(range(n_cores))],
                        ins=[ci.opt()], outs=[co.opt()])
                else:
                    nc.gpsimd.dma_start(out=co[:], in_=ci[:])
                return co

            def emit_gather_read(tag, co, out_w):
                ga = const.tile([128, out_w], F32, name=f"ga{tag}")
                nc.gpsimd.dma_start(
                    out=ga[:], in_=co[:].rearrange("(g p) f -> p g f", p=128))
                return ga

            def emit_gather_half(tag, cs):
                who = emit_who_half(tag, cs)
                co = emit_gather_start(tag, who[:], OB * 17)
                ga = emit_gather_read(tag, co, NB * 17)
                return who, ga

            # ---- phase B: hidden attention ----
            def finalize_head(h, ph):
                # 1/den = exp(-ln(den)) on ScalarE, straight from PSUM
                lnr = small.tile([65, OWN], F32, tag="lnr", name=f"lnr{h}")
                nc.scalar.activation(lnr[64:65, :], ph[64:65, :], AF.Ln)
                nc.scalar.activation(lnr[64:65, :], lnr[64:65, :], AF.Exp,
                                     scale=-1.0)
                rd = dram.tile([1, OWN], F32, name=f"rd{h}")
                nc.sync.dma_start(out=rd[:], in_=lnr[64:65, :])
                rb = small.tile([64, OWN], F32, tag="rb", name=f"rb{h}")
                nc.sync.dma_start(out=rb[:],
                                  in_=rd[0:1, :].to_broadcast([64, OWN]))
                tn = small.tile([64, OWN], F32, tag="tn", name=f"tn{h}")
                nc.vector.tensor_tensor(tn[:], ph[0:64, :], rb[:], MULT)
                m0 = small.tile([64, OWN], F32, tag="rb", name=f"m0{h}")
                nc.vector.tensor_scalar(m0[:], tn[:], 0.0, None,
                                        mybir.AluOpType.min)
                g_ = small.tile([64, OWN], F32, tag="g", name=f"g{h}")
                nc.scalar.activation(g_[:], m0[:], AF.Exp)
                slot = hT_all[(h % 2) * 64: (h % 2) * 64 + 64,
                              (h // 2) * OWN: (h // 2) * OWN + OWN]
                nc.vector.scalar_tensor_tensor(slot, g_[:], -1.0, tn[:], ADD, MAX)

            per = (NB4 + NG - 1) // NG
            prev = [None]
            gaA_ref = [None]
            coC_ref = [None]
            whoAC = const.tile([128, NB * 17], F32)
            s2oX = const.tile([128, NB], F32)
            for h in range(H):
                ph = psH.tile([65, OWN], F32, tag="ph", name=f"ph{h}")

                def pre_slab(g, h=h, ph=ph):
                    if h == 0:
                        # head 0: s-chunks two slabs ahead (slabs 0-1's are
                        # pre-emitted) and Wh chunks one slab ahead
                        for ch in range((g + 2) * 2, min((g + 3) * 2,
                                                         n // 512)):
                            emit_s_chunk(ch)
                        for j in range((g + 1) * per,
                                       min((g + 2) * per, NB4)):
                            emit_wh_chunk(0, j)
                    if h + 1 < H:
                        # emit next head's Wh chunks under this head's slabs
                        for j in range(g * per, min((g + 1) * per, NB4)):
                            emit_wh_chunk(h + 1, j)
                    if g == (1 if NG > 1 else 0) and prev[0] is not None:
                        # previous head's normalize/elu, off the critical path
                        finalize_head(h - 1, prev[0])
                    if h == 4 and g == 2:
                        # heads 0-3 finalized: gather their Wh_o contribution
                        # while heads 4-7 compute. Emitted mid-head so the
                        # pw matmuls/copies land mid-queue, not behind the
                        # whole head's work.
                        whoA = emit_who_half("A", [0, 1])
                        coA = emit_gather_start("A", whoA[:], OB * 17)
                        gaA_ref[0] = emit_gather_read("A", coA, NB * 17)
                    if h == 6 and g == 2:
                        # heads 4-5 finalized: start their gather during
                        # head 6/7; read + combine happen on the tail's Pool
                        # queue after the final collectives are triggered
                        whoC = emit_who_half("C", [2])
                        coC_ref[0] = emit_gather_start("C", whoC[:], OB * 17)

                attention(lambda jb, h=h: s2fg[jb // GROUP][:, (jb % GROUP) * 8 + h: (jb % GROUP) * 8 + h + 1],
                          s1b[h], wht[h], 65, 65, ph, f"h{h}",
                          pre_slab=pre_slab, d_leaky=(D_LEAKY, D_LEAKY_ALT),
                          halves=(h == H - 1))
                prev[0] = ph
            finalize_head(H - 1, prev[0])

            # ---- phase C: Wh_o (+s2o col) partial-sum gathers, pipelined:
            # c-blocks {0,1} gather at head 4, {2} at head 6 (pre-combined
            # on the Pool queue), so only c-block {3} (heads 6-7) plus a
            # tiny s2o-column gather sit on the critical tail. ----
            # s1ob depends only on hT_all: emit before the final collective
            s1ob = const.tile([128, OWN], BF16)
            ps1o = psM.tile([128, OWN], F32, tag="mm", name="ps1o")
            for c in range(KF):
                nc.tensor.matmul(ps1o[:], wo1[:, c * 128: (c + 1) * 128],
                                 hT_all[:, c * OWN: (c + 1) * OWN],
                                 start=(c == 0), stop=(c == KF - 1))
            nc.vector.tensor_copy(s1ob[:], ps1o[:])

            who17 = const.tile([128, NB * 17], BF16)
            nc.vector.memset(
                who17[:].rearrange("p (b w) -> p b w", w=17)[:, :, 16:17], 1.0)

            whoD = emit_who_half("D", [3])
            # tiny s2o-column gather triggered first: it unblocks the output
            # layer's elementwise work while the big Wh_o gather is in flight
            coS = emit_gather_start(
                "S", whoD[:].rearrange("p (b w) -> p b w", w=17)[:, :, 16:17],
                OB)
            coD = emit_gather_start("D", whoD[:], OB * 17)
            # C-half readback + A+C combines: the C collective completed
            # during head 7, so these run immediately, before the mini-
            # gather's wait blocks the Pool queue
            gaC = emit_gather_read("C", coC_ref[0], NB * 17)
            gaA = gaA_ref[0]
            nc.gpsimd.tensor_tensor(whoAC[:], gaA[:], gaC[:], ADD)
            nc.gpsimd.tensor_tensor(
                s2oX[:].rearrange("p (b w) -> p b w", w=1),
                gaA[:].rearrange("p (b w) -> p b w", w=17)[:, :, 16:17],
                gaC[:].rearrange("p (b w) -> p b w", w=17)[:, :, 16:17],
                ADD)
            s2obD = emit_gather_read("S", coS, NB)
            # s2o_all = A+C partial s2o + gathered D-half partials; on the
            # Pool queue so it lands right after the mini-gather without
            # blocking VectorE's queue
            s2oall = const.tile([128, NB], F32)
            nc.gpsimd.tensor_tensor(
                s2oall[:].rearrange("p (b w) -> p b w", w=1),
                s2oX[:].rearrange("p (b w) -> p b w", w=1),
                s2obD[:].rearrange("p (b w) -> p b w", w=1),
                ADD)
            gaD = emit_gather_read("D", coD, NB * 17)
            # fill the gather wait: z0 = nm + s1ob (+ the A/C-halves' s2o
            # partial, already local) for ALL output slabs; only the
            # D-half s2o delta waits on the mini-gather. Slabs 0-1 get a
            # dedicated pool tag so all four can be live at once.
            zpre = {}
            for g in range(min(2, NG)):
                zp = work.tile([128, GROUP * OWN], BF16, tag="z",
                               name=f"zpre{g}")
                nc.vector.tensor_tensor(
                    zp[:].rearrange("p (o w) -> p o w", w=OWN),
                    nm[:, g * GROUP * OWN: (g + 1) * GROUP * OWN]
                    .rearrange("p (o w) -> p o w", w=OWN),
                    s1ob[:].rearrange("p (o w) -> p o w", o=1)
                    .to_broadcast([128, GROUP, OWN]),
                    ADD)
                for q in range(GROUP):
                    jb = g * GROUP + q
                    nc.vector.tensor_scalar(
                        zp[:, q * OWN: (q + 1) * OWN],
                        zp[:, q * OWN: (q + 1) * OWN],
                        s2oX[:, jb: jb + 1], None, ADD)
                zpre[g] = zp
            # Wh_o assembly on the Pool queue: runs right after gaD's
            # readback in the same in-order queue, keeping VectorE free for
            # the output layer's z/leaky work
            whoall = const.tile([128, NB * 17], F32)
            nc.gpsimd.tensor_tensor(whoall[:], whoAC[:], gaD[:], ADD)
            nc.gpsimd.tensor_copy(
                who17[:].rearrange("p (b w) -> p b w", w=17)[:, :, 0:C],
                whoall[:].rearrange("p (b w) -> p b w", w=17)[:, :, 0:C])

            # ---- phase D: output attention ----
            po = psM.tile([128, OWN], F32, tag="mm", name="po")
            attention(lambda jb: s2oall[:, jb: jb + 1],
                      s1ob, who17, 17, 17, po, "o", premade=zpre,
                      d_leaky=(D_LEAKY_OUT,), gp_ztt=0,
                      s2col_pre=lambda jb: s2obD[:, jb: jb + 1],
                      halves=True)

            # ---- phase E: transpose, normalize, elu, log_softmax, store ----
            osb = const.tile([17, OWN], F32)
            nc.scalar.activation(osb[:], po[0:17, :], AF.Copy)
            ptr = psM.tile([128, OB * 17], F32, tag="mm", name="ptr")
            for tt in range(OB):
                nc.tensor.transpose(ptr[:, tt * 17: tt * 17 + 17],
                                    osb[0:17, tt * 128: (tt + 1) * 128],
                                    ident[0:17, 0:17])
            es = const.tile([128, OB * 17], F32)
            nc.vector.tensor_copy(es[:], ptr[:])
            rec4 = const.tile([128, OB], F32)
            nc.vector.reciprocal(
                rec4[:], es[:].rearrange("p (b w) -> p b w", w=17)[:, :, 16:17])
            # batched normalize / elu / log_softmax over all OB row-blocks
            t1 = const.tile([128, OB * C], F32)
            nc.vector.tensor_tensor(
                t1[:].rearrange("p (b w) -> p b w", w=C),
                es[:].rearrange("p (b w) -> p b w", w=17)[:, :, 0:C],
                rec4[:].rearrange("p (b w) -> p b w", w=1)
                .to_broadcast([128, OB, C]),
                MULT)
            m1 = const.tile([128, OB * C], F32)
            nc.vector.tensor_scalar(m1[:], t1[:], 0.0, None,
                                    mybir.AluOpType.min)
            g1 = const.tile([128, OB * C], F32)
            nc.scalar.activation(g1[:], m1[:], AF.Exp)
            e1all = const.tile([128, OB * C], F32)
            nc.vector.scalar_tensor_tensor(e1all[:], g1[:], -1.0, t1[:],
                                           ADD, MAX)
            sall = const.tile([128, OB], F32)
            final = const.tile([128, OB * C], F32)
            for tt in range(OB):
                ex = small.tile([128, C], F32, tag="ex", name=f"ex{tt}")
                nc.scalar.activation(ex[:], e1all[:, tt * C: (tt + 1) * C],
                                     AF.Exp, accum_out=sall[:, tt: tt + 1])
            lns = const.tile([128, OB], F32)
            nc.scalar.activation(lns[:], sall[:], AF.Ln)
            nc.vector.tensor_tensor(
                final[:].rearrange("p (b w) -> p b w", w=C),
                e1all[:].rearrange("p (b w) -> p b w", w=C),
                lns[:].rearrange("p (b w) -> p b w", w=1)
                .to_broadcast([128, OB, C]),
                mybir.AluOpType.subtract)
            nc.sync.dma_start(
                out=d_out[:].rearrange("(b p) c -> p b c", p=128),
                in_=final[:])

    nc.compile()
    return nc


def _pack_inputs(x, adj, W, a, W_out, a_out, n_cores=N_CORES):
    """Host-side shard + layout packing. Returns list of per-core in_maps."""
    n, f_in = x.shape
    OWN = n // n_cores
    NB = n // 128
    xf = np.asarray(x, np.float32)
    adj = np.asarray(adj)
    Wf = np.asarray(W, np.float32)
    af = np.asarray(a, np.float32)
    Wof = np.asarray(W_out, np.float32)
    aof = np.asarray(a_out, np.float32)

    # xT[p, k*n + m] = x[m, 128k+p]
    xT = xf.T.reshape(KF, 128, n).transpose(1, 0, 2).reshape(128, KF * n)
    xT = xT.astype(BF)
    w64 = (Wf.reshape(H, KF, 128, O).transpose(2, 0, 1, 3)
           .reshape(128, H * KF * O).astype(BF))
    wa1 = np.einsum("hfo,ho->hf", Wf, af[:, :O])  # [H, F]
    wa2 = np.einsum("hfo,ho->hf", Wf, af[:, O:])
    # waA[p, k*16 + m]: m<8 -> wa2[m], else wa1[m-8]
    waA = np.concatenate([wa2, wa1], axis=0)  # [16, F]
    waA = waA.T.reshape(KF, 128, 16).transpose(1, 0, 2).reshape(128, KF * 16)
    waA = waA.astype(BF)
    wo1 = Wof @ aof[:C]  # [F]
    wo1r = np.broadcast_to(
        wo1.reshape(KF, 128).T[:, :, None], (128, KF, 128)
    ).reshape(128, KF * 128).astype(BF)
    wot = (Wof.reshape(KF, 128, C).transpose(1, 0, 2)
           .reshape(128, KF * C).astype(BF))
    a2o = np.broadcast_to(aof[C:], (128, C)).astype(np.float32).copy()

    in_maps = []
    for core in range(n_cores):
        rows = slice(core * OWN, (core + 1) * OWN)
        xo = (xf[rows].T.reshape(KF, 128, OWN).transpose(1, 0, 2)
              .reshape(128, KF * OWN).astype(BF))
        nmT = np.where(adj[rows].T > 0, np.float32(0), np.float32(NEG))
        nmT = (nmT.reshape(NB, 128, OWN).transpose(1, 0, 2)
               .reshape(128, NB * OWN).astype(BF))
        in_maps.append({
            "xT": xT, "xo": xo, "w64": w64, "waA": waA, "wo1r": wo1r,
            "nmT": nmT, "wot": wot, "a2o": a2o,
        })
    return in_maps


_NC_CACHE = {}


def _get_nc(n_cores=N_CORES, n=N):
    key = (n_cores, n)
    if key not in _NC_CACHE:
        _NC_CACHE[key] = _build_nc(n_cores, n)
    return _NC_CACHE[key]


def kernel(x, adj, W, a, W_out, a_out):
    nc = _get_nc()
    in_maps = _pack_inputs(x, adj, W, a, W_out, a_out)
    res = run_bass_kernel_spmd(nc, in_maps, list(range(N_CORES)))
    out = np.concatenate([res.results[c]["out"] for c in range(N_CORES)], axis=0)
    return out.astype(np.float32)
